# revision 31
# baseline (speedup 1.0000x reference)
"""Multi-head causal attention (B=4, T=2048, D=1024, H=16, HD=64) on 8 TRN2 cores.

Sharding: core = 2*b + g  (b in 0..3 batch, g in 0..1 head-group of 8 heads;
tensor-parallel on the QKV output columns / Wo rows).
Each core computes, for its (b, g):
  QT,KT = Wq_g^T x_b^T + b    layout [512, T] (d on partitions), bf16
  V     = x_b Wv_g + bv       layout [T, 512] (t on partitions), bf16, with a
                              ones column appended per head (softmax colsums)
  per head h: S^T = K_h Q_h^T (scale 1/8), E = exp(S^T) causal-masked (bf16),
  AV matmul gives unnormalized ctx^T [64, tq] + colsums row; cps is copied to
  SBUF immediately (frees PSUM), normalized off the PE critical path, then
  partial out = ctx @ Wo_g, DMA'd out as bf16.
Host upcasts and sums the two partials per batch element (+ b_o).

Engines execute in program order, so the emission is software-pipelined:
projection matmuls for strip s+1 and the output projection of strip s-1 are
woven between the attention score/exp groups of strip s, filling the PE
during ACT-bound (softmax) stretches.  DMAs are coarse (one tile per weight
matrix / x strip) to amortize the ~625ns/DMA HWDGE overhead, ordered so x
strip 0 and wq/wk land first and wo last.
"""
import numpy as np
from collections import deque
from contextlib import ExitStack

import concourse.bacc as bacc
import concourse.bass as bass
import concourse.mybir as mybir
import concourse.tile as tile

F32 = mybir.dt.float32
F32R = mybir.dt.float32r
BF16 = mybir.dt.bfloat16
FP8 = mybir.dt.float8e4
AF = mybir.ActivationFunctionType
DR = mybir.MatmulPerfMode.DoubleRow
USE_FP8 = False  # fp8e4m3 DoubleRow AV on full blocks (2x PE rate there)

B, T, DIN, DOUT, H = 4, 2048, 1024, 1024, 16
DL = 512          # local d_out slice (8 heads)
NH = 8            # local heads
S = 512           # tq strip width
NS = T // S       # 4 strips
KC = DIN // 128   # 8 k-chunks for projections
CC = DL // 128    # 4 dlocal chunks (head pairs)
NT = T // 128     # 16 tk tiles
VW = NH * 65      # V' width: 8 heads x (64 + ones col)

# Diagonal-region packing per (strip, head): the 4 partial blocks
# j = 4*s + db cover strip-local tq columns [128*db, 512), width w = 512-128*db.
#   tile1: db0 at cols 0:512, db1 at 512:896 ; tile2: db2 at 0:256, db3 at 256:384
DIAG = [[(0, 0, 512), (1, 512, 384)], [(2, 0, 256), (3, 256, 128)]]


def _build_nc(reps=1):
    nc = bacc.Bacc("TRN2", target_bir_lowering=False, debug=False,
                   enable_asserts=False)
    xT_d = nc.dram_tensor("xT", [DIN, T], BF16, kind="ExternalInput").ap()
    wq_d = nc.dram_tensor("wq", [DIN, DL], BF16, kind="ExternalInput").ap()
    wk_d = nc.dram_tensor("wk", [DIN, DL], BF16, kind="ExternalInput").ap()
    wv_d = nc.dram_tensor("wv", [DIN, DL], BF16, kind="ExternalInput").ap()
    bqk_d = nc.dram_tensor("bqk", [128, 2 * CC], F32, kind="ExternalInput").ap()
    bv_d = nc.dram_tensor("bv", [1, DL], F32, kind="ExternalInput").ap()
    wo_d = nc.dram_tensor("wo", [DL, DOUT], BF16, kind="ExternalInput").ap()
    tri_d = nc.dram_tensor("tri", [128, 128], BF16, kind="ExternalInput").ap()
    out_d = nc.dram_tensor("out", [T, DOUT], BF16, kind="ExternalOutput").ap()

    with tile.TileContext(nc) as tc:
      for _rep in range(reps):
        with ExitStack() as ctx:
          const = ctx.enter_context(tc.tile_pool(name="const", bufs=1))
          ktp = ctx.enter_context(tc.tile_pool(name="ktp", bufs=1))
          vpp = ctx.enter_context(tc.tile_pool(name="vpp", bufs=1))
          ctxp = ctx.enter_context(tc.tile_pool(name="ctxp", bufs=4))
          # PSUM banks: pj 2x[128,512]=2; pp 2x[128,1024]=4; cp 2x[65,512]=2
          pj = ctx.enter_context(tc.tile_pool(name="pj", bufs=2, space="PSUM"))
          pp = ctx.enter_context(tc.tile_pool(name="pp", bufs=2, space="PSUM"))
          cp = ctx.enter_context(tc.tile_pool(name="cp", bufs=1, space="PSUM"))

          with tc.tile_pool(name="wts", bufs=1) as wts, \
               tc.tile_pool(name="xsp", bufs=2) as xsp, \
               tc.tile_pool(name="qsp", bufs=2) as qsp, \
               tc.tile_pool(name="ep", bufs=2) as ep, \
               tc.tile_pool(name="csp", bufs=2) as csp, \
               tc.tile_pool(name="rp", bufs=2) as rp:

              xs_tiles = {}

              def load_x(s, chunked=False):
                  xa = xsp.tile([128, KC * S], BF16, tag="xs", name="xa")
                  if chunked:
                      for k2 in range(KC // 2):
                          nc.sync.dma_start(
                              xa[:, k2 * 2 * S:(k2 + 1) * 2 * S]
                              .rearrange("p (k t) -> p k t", k=2),
                              xT_d[k2 * 256:(k2 + 1) * 256,
                                   s * S:(s + 1) * S]
                              .rearrange("(k p) t -> p k t", p=128))
                  else:
                      nc.sync.dma_start(
                          xa.rearrange("p (k t) -> p k t", k=KC),
                          xT_d[:, s * S:(s + 1) * S]
                          .rearrange("(k p) t -> p k t", p=128))
                  xs_tiles[s] = xa

              # ---- startup: interleave x strip 0 / wq / wk 2k-chunks so the
              # first projection starts after ~1 chunk of each ----
              xa0 = xsp.tile([128, KC * S], BF16, tag="xs", name="xa")
              xs_tiles[0] = xa0
              wq_a = wts.tile([128, KC * DL], BF16, name="wq_a")
              wk_a = wts.tile([128, KC * DL], BF16, name="wk_a")
              for k2 in range(KC // 2):
                  sl2 = slice(k2 * 256, (k2 + 1) * 256)
                  nc.sync.dma_start(
                      xa0[:, k2 * 2 * S:(k2 + 1) * 2 * S]
                      .rearrange("p (k t) -> p k t", k=2),
                      xT_d[sl2, 0:S].rearrange("(k p) t -> p k t", p=128))
                  nc.sync.dma_start(
                      wq_a[:, k2 * 2 * DL:(k2 + 1) * 2 * DL]
                      .rearrange("p (k d) -> p k d", k=2),
                      wq_d[sl2, :].rearrange("(k p) d -> p k d", p=128))
                  nc.sync.dma_start(
                      wk_a[:, k2 * 2 * DL:(k2 + 1) * 2 * DL]
                      .rearrange("p (k d) -> p k d", k=2),
                      wk_d[sl2, :].rearrange("(k p) d -> p k d", p=128))

              # ---- small constants ----
              bqk_t = const.tile([128, 2 * CC], F32)
              nc.sync.dma_start(bqk_t[:], bqk_d[:])
              bv_f = wts.tile([1, DL], F32)
              nc.sync.dma_start(bv_f[:], bv_d[:])
              tri_t = const.tile([128, 128], BF16)
              nc.sync.dma_start(tri_t[:], tri_d[:])

              wv_a = wts.tile([128, KC * DL], BF16, name="wv_a")
              nc.sync.dma_start(
                  wv_a.rearrange("p (k d) -> p k d", k=KC),
                  wv_d.rearrange("(k p) d -> p k d", p=128))
              wo_a = wts.tile([128, CC * DOUT], BF16, name="wo_a")
              nc.sync.dma_start(
                  wo_a.rearrange("p (c d) -> p c d", c=CC),
                  wo_d.rearrange("(c p) d -> p c d", p=128))

              bvb = wts.tile([128, DL], F32)
              nc.gpsimd.partition_broadcast(bvb[:], bv_f[:])
              onecols_f = const.tile([128, NH], BF16)
              nc.vector.memset(onecols_f[:], 1.0)

              # ---- persistent tensors ----
              kt = [ktp.tile([128, T], BF16, name=f"kt{c}") for c in range(CC)]
              vp = [vpp.tile([128, VW], BF16, name=f"vp{j}") for j in range(NT)]
              for j in range(NT):
                  nc.vector.tensor_copy(
                      vp[j].rearrange("p (h w) -> p h w", w=65)[:, :, 64:65],
                      onecols_f.rearrange("p (h o) -> p h o", o=1))
              # fp8 copies of V', paired tk-blocks for DoubleRow AV on the
              # full (off-diagonal) blocks; head slots padded to 66 so the
              # weight AP's subtile step stays 16B-aligned
              vp2 = []
              if USE_FP8:
                  vp2 = [vpp.tile([128, 2 * 66 * NH], FP8, name=f"vp2_{j2}")
                         for j2 in range(NT // 2)]
                  for j2 in range(NT // 2):
                      nc.vector.memset(
                          vp2[j2].rearrange("p (i h w) -> p i h w", i=2, w=66)
                          [:, :, :, 64:65], 1.0)

              qt_tiles = {}
              ctxt_tiles = {}

              def proj_units(s):
                  xa = xs_tiles[s]
                  units = []

                  def q_unit(c=0, s=s, xa=xa):
                      pq = pj.tile([128, S], F32, tag="pj", name="pq")
                      for k in range(KC):
                          nc.tensor.matmul(
                              pq[:],
                              wq_a[:, k * DL + c * 128:k * DL + (c + 1) * 128],
                              xa[:, k * S:(k + 1) * S],
                              start=(k == 0), stop=(k == KC - 1))
                      qs = qsp.tile([128, S], BF16, tag=f"qt{c}", name="qs")
                      nc.vector.tensor_scalar_add(qs[:], pq[:],
                                                  bqk_t[:, c:c + 1])
                      qt_tiles[(s, c)] = qs

                  def k_unit(c=0, s=s, xa=xa):
                      pk = pj.tile([128, S], F32, tag="pj", name="pk")
                      for k in range(KC):
                          nc.tensor.matmul(
                              pk[:],
                              wk_a[:, k * DL + c * 128:k * DL + (c + 1) * 128],
                              xa[:, k * S:(k + 1) * S],
                              start=(k == 0), stop=(k == KC - 1))
                      nc.vector.tensor_scalar_add(
                          kt[c][:, s * S:(s + 1) * S], pk[:],
                          bqk_t[:, CC + c:CC + c + 1])

                  def v_unit(m=0, s=s, xa=xa):
                      pv = pj.tile([128, DL], F32, tag="pj", name="pv")
                      for k in range(KC):
                          nc.tensor.matmul(
                              pv[:], xa[:, k * S + m * 128:k * S + (m + 1) * 128],
                              wv_a[:, k * DL:(k + 1) * DL],
                              start=(k == 0), stop=(k == KC - 1))
                      j = s * 4 + m
                      nc.vector.tensor_add(
                          vp[j].rearrange("p (h w) -> p h w", w=65)[:, :, 0:64],
                          pv.rearrange("p (h w) -> p h w", w=64),
                          bvb.rearrange("p (h w) -> p h w", w=64))
                      if USE_FP8:
                          # fp8 copy (paired layout) for later strips'
                          # full-block DoubleRow AV; off the critical path
                          nc.vector.tensor_add(
                              vp2[j // 2].rearrange("p (i h w) -> p i h w",
                                                    i=2, w=66)
                              [:, j % 2, :, 0:64],
                              pv.rearrange("p (h w) -> p h w", w=64),
                              bvb.rearrange("p (h w) -> p h w", w=64))

                  for c in range(CC):
                      units.append(lambda c=c: q_unit(c))
                      units.append(lambda c=c: k_unit(c))
                  for m in range(4):
                      units.append(lambda m=m: v_unit(m))
                  return units

              def outproj_units(s):
                  units = []

                  def o_unit(ml, s=s):
                      ctxt = ctxt_tiles[s]
                      m = 4 * s + ml
                      ot = rp.tile([128, DOUT], BF16, tag="ot", name="ot",
                                   bufs=2)
                      for n in range(2):
                          po = pj.tile([128, 512], F32, tag="pj", name="po")
                          for c in range(CC):
                              nc.tensor.matmul(
                                  po[:], ctxt[c][:, ml * 128:(ml + 1) * 128],
                                  wo_a[:, c * DOUT + n * 512:
                                       c * DOUT + (n + 1) * 512],
                                  start=(c == 0), stop=(c == CC - 1))
                          nc.vector.tensor_copy(ot[:, n * 512:(n + 1) * 512],
                                                po[:])
                          # per-half DMA: bytes start moving while the other
                          # half is still being copied (shortens the drain)
                          nc.sync.dma_start(
                              out_d[m * 128:(m + 1) * 128,
                                    n * 512:(n + 1) * 512],
                              ot[:, n * 512:(n + 1) * 512])

                  for ml in range(4):
                      units.append(lambda ml=ml: o_unit(ml))
                  return units

              for s in range(NS):
                  if s == 0:
                      for u in proj_units(0):
                          u()
                  # fillers: projections for s+1 early; out-projections
                  # deferred as late as ctxp rotation allows (strip 3 is the
                  # longest ACT-bound stretch and has no proj work left)
                  fillers = deque()
                  if s + 1 < NS:
                      load_x(s + 1)
                      fillers.extend(proj_units(s + 1))
                  if s == 3:
                      fillers.extend(outproj_units(0))
                      fillers.extend(outproj_units(1))
                      fillers.extend(outproj_units(2))
                  npts = 8 * s + 8
                  state = {"q": 0.0, "step": len(fillers) / npts}

                  def point():
                      state["q"] += state["step"]
                      while state["q"] >= 1.0 and fillers:
                          fillers.popleft()()
                          state["q"] -= 1.0

                  # ---- attention for query strip s ----
                  qt = [qt_tiles[(s, c)] for c in range(CC)]
                  ctxt = [ctxp.tile([128, S], BF16, tag=f"ctx{c}", name="cx")
                          for c in range(CC)]
                  ctxt_tiles[s] = ctxt
                  nf = 4 * s
                  for c in range(CC):
                      cps = [cp.tile([65, S], F32, tag="cA", name="cA"),
                             cp.tile([65, S], F32, tag="cB", name="cB")]
                      # full blocks, two per PSUM tile; both head-halves'
                      # score matmuls batched (64-row tiling mode) before the
                      # filler point so mode switches stay one-per-group
                      for grp in range(nf // 2):
                          sts = []
                          for hi, boff in enumerate((0, 64)):
                              st = pp.tile([128, 1024], F32, tag="s", name="sS")
                              for i in range(2):
                                  kb = 2 * grp + i
                                  nc.tensor.matmul(
                                      st[:, i * 512:(i + 1) * 512],
                                      kt[c][boff:boff + 64,
                                            kb * 128:(kb + 1) * 128],
                                      qt[c][boff:boff + 64, :],
                                      start=True, stop=True,
                                      tile_position=(boff, 0))
                              sts.append(st)
                          etf = []
                          for hi in range(2):
                              et = ep.tile([128, 1024], FP8 if USE_FP8 else BF16,
                                           tag="e", name="eS")
                              nc.scalar.activation(et[:], sts[hi][:], AF.Exp,
                                                   scale=0.125)
                              etf.append(et)
                          point()
                          for hi in range(2):
                              h = 2 * c + hi
                              if USE_FP8:
                                  nc.tensor.matmul(
                                      cps[hi][:],
                                      vp2[grp].rearrange("p (i c2) -> p i c2",
                                                         i=2)
                                      [:, :, h * 66:h * 66 + 65],
                                      etf[hi].rearrange("p (i t) -> p i t", i=2),
                                      start=(grp == 0), stop=False,
                                      perf_mode=DR)
                              else:
                                  for i in range(2):
                                      kb = 2 * grp + i
                                      nc.tensor.matmul(
                                          cps[hi][:],
                                          vp[kb][:, h * 65:h * 65 + 65],
                                          etf[hi][:, i * 512:(i + 1) * 512],
                                          start=(kb == 0), stop=False)
                      # diagonal region
                      for hi, boff in enumerate((0, 64)):
                          h = 2 * c + hi
                          stts = []
                          for blks in DIAG:
                              stt = pp.tile([128, 1024], F32, tag="s", name="sD")
                              for db, off, w in blks:
                                  j = nf + db
                                  nc.tensor.matmul(
                                      stt[:, off:off + w],
                                      kt[c][boff:boff + 64,
                                            j * 128:(j + 1) * 128],
                                      qt[c][boff:boff + 64,
                                            128 * db:128 * db + w],
                                      start=True, stop=True,
                                      tile_position=(boff, 0))
                              stts.append(stt)
                          ets = []
                          for stt, blks in zip(stts, DIAG):
                              tot = sum(w for _, _, w in blks)
                              et = ep.tile([128, 1024], BF16, tag="e", name="eD")
                              nc.scalar.activation(et[:, 0:tot], stt[:, 0:tot],
                                                   AF.Exp, scale=0.125)
                              ets.append(et)
                          point()
                          for et, blks in zip(ets, DIAG):
                              for db, off, w in blks:
                                  nc.vector.tensor_mul(et[:, off:off + 128],
                                                       et[:, off:off + 128],
                                                       tri_t[:])
                          for et, blks in zip(ets, DIAG):
                              for db, off, w in blks:
                                  j = nf + db
                                  nc.tensor.matmul(
                                      cps[hi][:, S - w:S],
                                      vp[j][:, h * 65:h * 65 + 65],
                                      et[:, off:off + w],
                                      start=(j == 0), stop=(db == 3))
                      # copy ctx+colsums off PSUM immediately (frees the cp
                      # banks for the next head pair), normalize from SBUF.
                      # The very last head pair of the kernel has no successor
                      # waiting on the banks: normalize straight from PSUM to
                      # shorten the chain into the final output projection.
                      last_grp = (s == NS - 1 and c == CC - 1)
                      for hi in range(2):
                          if last_grp:
                              src = cps[hi]
                          else:
                              src = csp.tile([65, S], F32, tag="cs", name="cs")
                              nc.vector.tensor_copy(src[:], cps[hi][:])
                          rec = rp.tile([1, S], F32, tag="rec", name="rec")
                          nc.vector.reciprocal(rec[:], src[64:65, :])
                          rbc = rp.tile([64, S], F32, tag="rbc", name="rbc")
                          nc.gpsimd.partition_broadcast(rbc[:], rec[:])
                          nc.vector.tensor_mul(
                              ctxt[c][hi * 64:hi * 64 + 64, :],
                              src[0:64, :], rbc[:])

                  while fillers:
                      fillers.popleft()()

              for u in outproj_units(NS - 1):
                  u()

    nc.compile()
    return nc


_NC = None


def _get_nc():
    global _NC
    if _NC is None:
        _NC = _build_nc()
    return _NC


def _bf16(a):
    import ml_dtypes
    return np.ascontiguousarray(a).astype(ml_dtypes.bfloat16)


def make_in_maps(x, w_q, b_q, w_k, b_k, w_v, b_v, w_o, b_o):
    tri = np.triu(np.ones((128, 128), dtype=np.float32))
    in_maps = []
    for core in range(8):
        b, g = core // 2, core % 2
        sl = slice(g * DL, (g + 1) * DL)
        bqk = np.concatenate([np.asarray(b_q[sl]).reshape(CC, 128).T,
                              np.asarray(b_k[sl]).reshape(CC, 128).T],
                             axis=1).astype(np.float32)
        in_maps.append({
            "xT": _bf16(x[b].T),
            "wq": _bf16(w_q[:, sl]),
            "wk": _bf16(w_k[:, sl]),
            "wv": _bf16(w_v[:, sl]),
            "bqk": np.ascontiguousarray(bqk),
            "bv": np.ascontiguousarray(b_v[sl].reshape(1, DL)).astype(np.float32),
            "wo": _bf16(w_o[sl, :]),
            "tri": _bf16(tri),
        })
    return in_maps


def kernel(x, w_q, b_q, w_k, b_k, w_v, b_v, w_o, b_o):
    from concourse.bass_utils import run_bass_kernel_spmd
    nc = _get_nc()
    in_maps = make_in_maps(np.asarray(x), np.asarray(w_q), np.asarray(b_q),
                           np.asarray(w_k), np.asarray(b_k), np.asarray(w_v),
                           np.asarray(b_v), np.asarray(w_o), np.asarray(b_o))
    res = run_bass_kernel_spmd(nc, in_maps, core_ids=list(range(8)))
    outs = [np.asarray(r["out"]).astype(np.float32) for r in res.results]
    bo = np.asarray(b_o).reshape(1, DOUT).astype(np.float32)
    full = np.stack([outs[2 * b] + outs[2 * b + 1] + bo for b in range(B)])
    return full.astype(np.float32)
